# revision 13
# baseline (speedup 1.0000x reference)
"""ControlNorm1D online-normalization forward, Trainium2 Bass kernel.

Math (per feature column l, sequential over rows t):
    scale_t = sqrt(v_t + eps);  d_t = x_t - mu_t;  out_t = d_t / scale_t
    v_{t+1}  = a*v_t + a*(1-a)*d_t^2
    mu_{t+1} = a*mu_t + (1-a)*x_t

Both mu and v are first-order linear recurrences with constant decay, so blocks
of rows become matmuls against constant triangular coefficient matrices.

Blocking: rows are processed in pairs of 127-row blocks (254-row super-blocks).
For a pair with SBUF tiles R0/R1 ([128, 512]; partition 1+t = x row, partition 0
of R0 = virtual carry row -mu0):
    psD0 = LD_ev^T R0                 : partitions 1+t = d_t (t=0..126)
    psD1 = LD_x^T R0 + LD_od^T R1     : partitions 1+t = d_{127+t}; partition 0
                                        = -mu0_next (carry for the next pair)
    d2   = Square(sig*psD)  (ACT, one op over the psD0|psD1 pair, sig^2=a(1-a))
    psV0 = LV_ev^T d20
    psV1 = LV_x^T d20 + LV_od^T d21   : partition 0 = v0_next
    r    = Rsqrt(psV + eps) (ACT, one op over the psV0|psV1 pair)
    out  = psD * r          (DVE, one op over the pair)
Carries between pairs are two single-row PSUM->SBUF copies.

Perf structure vs the f32 baseline (244us):
  - W work buffer, x upload and out download are bf16 (DMA bytes halved;
    tolerance is 2e-2, measured error stays ~5e-3).
  - psD0/psD1 (and psV0/psV1) live in one [128, 1024] PSUM tile spanning two
    adjacent banks, so the ACT square, ACT rsqrt and DVE multiply each run
    once per PAIR (FD=1024) instead of once per block -- amortizes the
    ~350cyc ACT / ~120cyc DVE fixed overheads (engines run 1 elem/lane/cyc).
  - a(1-a) is folded into the ACT Square scale so LV coefficients are O(1).
  - Stationaries stay f32r (mixed f32r x bf16 matmul is legal, 1-pass).
  - All big DMAs go over HWDGE (nc.sync), chunked; small first chunk so
    compute starts early; out-chunks stream back as pairs complete.

The feature dim L=4096 is sharded across 8 cores (512 each, no cross-core
communication).  Host-side, each core's x shard is PRE-TILED to the exact SBUF
layout [128 partitions, 65 blocks * 512] so device DMAs have one big contiguous
run per partition.  Outputs are written back over the same SBUF buffer and
un-pretiled on host.
"""

import numpy as np
import ml_dtypes

AFWD = 0.999
EPS = 1e-5
N_ROWS = 8192
L_FULL = 4096
N_CORES = 8
LC = L_FULL // N_CORES  # 512 features per core
B = 127                 # rows per block (partition 1+t holds row t)
NBLK = 65               # 64 full blocks + 1 short (64 rows)
NPAIR = 32              # paired blocks; block 64 handled as a single tail

_f32 = np.float32
_bf16 = ml_dtypes.bfloat16
SQ_SCALE = float(np.sqrt(AFWD * (1.0 - AFWD)))  # folded into ACT Square


def _tri(me, ve):
    a = AFWD
    L = np.zeros((128, 128))
    for t in range(127):
        for s in range(127):
            if s == t:
                L[1 + s, 1 + t] += me
            if s < t:
                L[1 + s, 1 + t] += ve * a ** (t - 1 - s)
    return L


def _build_mats():
    a = AFWD
    LD_ev = _tri(1.0, -(1 - a))
    LV_ev = _tri(0.0, 1.0)          # d2 rows carry a(1-a)d^2 via SQ_SCALE
    for t in range(127):
        LD_ev[0, 1 + t] = a**t
        LV_ev[0, 1 + t] = a**t
    LD_x = np.zeros((128, 128))
    LV_x = np.zeros((128, 128))
    for t in range(127):
        LD_x[0, 1 + t] = a ** (127 + t)
        LV_x[0, 1 + t] = a ** (127 + t)
        for s in range(127):
            LD_x[1 + s, 1 + t] = -(1 - a) * a ** (127 + t - 1 - s)
            LV_x[1 + s, 1 + t] = a ** (127 + t - 1 - s)
    LD_x[0, 0] = a**254
    LV_x[0, 0] = a**254
    for s in range(127):
        LD_x[1 + s, 0] = -(1 - a) * a ** (253 - s)
        LV_x[1 + s, 0] = a ** (253 - s)
    LD_od = _tri(1.0, -(1 - a))
    LV_od = _tri(0.0, 1.0)
    for s in range(127):
        LD_od[1 + s, 0] = -(1 - a) * a ** (126 - s)
        LV_od[1 + s, 0] = a ** (126 - s)
    LD_s = _tri(1.0, -(1 - a))
    LV_s = _tri(0.0, 1.0)
    for t in range(127):
        LD_s[0, 1 + t] = a**t
        LV_s[0, 1 + t] = a**t
    mats = [LD_ev, LD_x, LD_od, LV_ev, LV_x, LV_od, LD_s, LV_s]
    return np.stack([m.astype(_f32) for m in mats])  # [8, 128, 128]


def _pretile(x_c, m_c):
    """[8192, LC] -> [128, NBLK*LC] bf16: partition 1+t of block-slice i =
    row i*127+t.  Partition 0 of block 0 carries the virtual row -m."""
    xp = np.zeros((128, NBLK * LC), _f32)
    full = x_c[: 64 * B].reshape(64, B, LC).transpose(1, 0, 2)  # [127, 64, LC]
    xp[1:128, : 64 * LC] = full.reshape(B, 64 * LC)
    xp[1:65, 64 * LC :] = x_c[64 * B :]
    xp[0, :LC] = -m_c
    return xp.astype(_bf16)


def _unpretile(op):
    """Inverse of _pretile for the (bf16) output buffer."""
    op = np.asarray(op).astype(_f32)
    out = np.empty((N_ROWS, LC), _f32)
    out[: 64 * B] = (
        op[1:128, : 64 * LC].reshape(B, 64, LC).transpose(1, 0, 2).reshape(-1, LC)
    )
    out[64 * B :] = op[1:65, 64 * LC :]
    return out


_PROGRAM_CACHE: dict = {}


def _raw_act(eng, out, in_, func, bias_ap, scale, mybir):
    ins = [
        eng.lower_ap(in_),
        eng.lower_ap(bias_ap),
        mybir.ImmediateValue(dtype=mybir.dt.float32, value=float(scale)),
        mybir.ImmediateValue(dtype=mybir.dt.float32, value=0.0),
    ]
    return eng.add_instruction(
        mybir.InstActivation(
            name=eng.bass.get_next_instruction_name(),
            func=func,
            ins=ins,
            outs=[eng.lower_ap(out)],
        )
    )


def _build_program():
    if "nc" in _PROGRAM_CACHE:
        return _PROGRAM_CACHE["nc"]

    import concourse.bacc as bacc
    import concourse.tile as tile
    from concourse import mybir

    nc = bacc.Bacc(
        "TRN2",
        target_bir_lowering=False,
        debug=False,
        enable_asserts=False,
        num_devices=N_CORES,
    )
    f32 = mybir.dt.float32
    bf16 = mybir.dt.bfloat16
    f32r = mybir.dt.float32r

    xp_d = nc.dram_tensor("xp", [128, NBLK * LC], bf16, kind="ExternalInput").ap()
    var_d = nc.dram_tensor("var", [LC], f32, kind="ExternalInput").ap()
    mats_d = nc.dram_tensor("mats", [8, 128, 128], bf16, kind="ExternalInput").ap()
    matsv_d = nc.dram_tensor("matsv", [8, 128, 128], f32, kind="ExternalInput").ap()
    op_d = nc.dram_tensor("op", [128, NBLK * LC], bf16, kind="ExternalOutput").ap()

    # in-chunks: small first chunk so compute starts early
    chunks = []
    b0 = 0
    for cb in (4, 12, 14, 14, 14, 7):
        b1 = min(b0 + cb, NBLK)
        if b1 > b0:
            chunks.append((b0, b1))
        b0 = b1

    with tile.TileContext(nc) as tc:
        with (
            tc.tile_pool(name="consts", bufs=1) as consts,
            tc.tile_pool(name="work", bufs=1) as work,
            tc.tile_pool(name="d2", bufs=2) as d2_pool,
            tc.tile_pool(name="rs", bufs=2) as r_pool,
            tc.tile_pool(name="psD", bufs=2, space="PSUM") as psD_pool,
            tc.tile_pool(name="psV", bufs=2, space="PSUM") as psV_pool,
        ):
            mat_tiles = []
            for mi in range(8):
                if mi in (3, 4, 5, 7):  # LV_ev, LV_x, LV_od, LV_s stay 32-bit
                    mt = consts.tile([128, 128], f32r, tag=f"mat{mi}")
                    nc.gpsimd.dma_start(out=mt[:], in_=matsv_d[mi, :, :])
                else:
                    mt = consts.tile([128, 128], bf16, tag=f"mat{mi}")
                    nc.sync.dma_start(out=mt[:], in_=mats_d[mi, :, :])
                mat_tiles.append(mt)
            eps_t = consts.tile([128, 1], f32)
            nc.vector.memset(eps_t[:], EPS)
            zero_t = consts.tile([128, 1], f32)
            nc.vector.memset(zero_t[:], 0.0)

            # prime both ACT table sets during the initial DMA wait
            warm_t = consts.tile([128, 1], f32)
            _raw_act(nc.scalar, warm_t[:], eps_t[:],
                     mybir.ActivationFunctionType.Square,
                     zero_t[0:128, 0:1], 1.0, mybir)
            _raw_act(nc.scalar, warm_t[:], eps_t[:],
                     mybir.ActivationFunctionType.Rsqrt,
                     zero_t[0:128, 0:1], 1.0, mybir)

            W = work.tile([128, NBLK * LC], bf16)

            # bulk x traffic via SWDGE (gpsimd): measured ~386 GB/s aggregate
            # vs HWDGE's serialized trickle on this access pattern
            for (b0, b1) in chunks:
                nc.gpsimd.dma_start(
                    out=W[:, b0 * LC : b1 * LC], in_=xp_d[:, b0 * LC : b1 * LC]
                )

            v0_t = consts.tile([1, LC], f32)
            nc.sync.dma_start(out=v0_t[:], in_=var_d[None, :])

            LD_ev, LD_x, LD_od, LV_ev, LV_x, LV_od, LD_s, LV_s = (
                mt[:] for mt in mat_tiles
            )

            HC = LC // 2  # 256-column chain width

            class Chain:
                pass

            chains = []
            for c in range(2):
                ch = Chain()
                ch.c = c
                ch.prev_D = None
                ch.prev_V = None
                ch.prev_r = None
                ch.prev_sp = None
                chains.append(ch)

            out_chunks = []
            b0 = 0
            for cb in (8, 8, 8, 8, 8, 8, 8, 6, 3):
                b1 = min(b0 + cb, NBLK)
                if b1 > b0:
                    out_chunks.append((b0, b1))
                b0 = b1
            out_chunk_done = [False] * len(out_chunks)
            mul_done_pairs = [0, 0]

            def emit_out_dma(upto_block):
                for ci, (b0, b1) in enumerate(out_chunks):
                    if not out_chunk_done[ci] and b1 <= upto_block:
                        nc.gpsimd.dma_start(
                            out=op_d[:, b0 * LC : b1 * LC],
                            in_=W[:, b0 * LC : b1 * LC],
                        )
                        out_chunk_done[ci] = True

            def col(blk, c, n=1):
                # chain-c columns of pretiled block blk (n consecutive... n unused)
                return slice(blk * LC + c * HC, blk * LC + (c + 1) * HC)

            for i in range(NPAIR):
                for ch in chains:
                    c = ch.c
                    s0 = col(2 * i, c)
                    s1 = col(2 * i + 1, c)
                    R0 = W[:, s0]
                    R1 = W[:, s1]

                    if i > 0:
                        nc.vector.tensor_copy(
                            out=W[0:1, s0], in_=ch.prev_D[0:1, HC : 2 * HC]
                        )

                    psD = psD_pool.tile([128, 2 * HC], f32, tag=f"psD{c}")
                    nc.tensor.matmul(psD[:, 0:HC], LD_ev, R0, start=True, stop=True)
                    nc.tensor.matmul(
                        psD[:, HC : 2 * HC], LD_x, R0, start=True, stop=False
                    )
                    nc.tensor.matmul(
                        psD[:, HC : 2 * HC], LD_od, R1, start=False, stop=True
                    )

                    d2 = d2_pool.tile([128, 2 * HC], f32r, tag=f"d2{c}")
                    _raw_act(
                        nc.scalar,
                        d2[:, :],
                        psD[:, :],
                        mybir.ActivationFunctionType.Square,
                        zero_t[0:128, 0:1],
                        SQ_SCALE,
                        mybir,
                    )
                    if i == 0:
                        nc.vector.tensor_copy(
                            out=d2[0:1, 0:HC], in_=v0_t[0:1, c * HC : (c + 1) * HC]
                        )
                    else:
                        nc.vector.tensor_copy(
                            out=d2[0:1, 0:HC], in_=ch.prev_V[0:1, HC : 2 * HC]
                        )

                    psV = psV_pool.tile([128, 2 * HC], f32, tag=f"psV{c}")
                    nc.tensor.matmul(
                        psV[:, 0:HC], LV_ev, d2[:, 0:HC], start=True, stop=True
                    )
                    nc.tensor.matmul(
                        psV[:, HC : 2 * HC], LV_x, d2[:, 0:HC], start=True, stop=False
                    )
                    nc.tensor.matmul(
                        psV[:, HC : 2 * HC],
                        LV_od,
                        d2[:, HC : 2 * HC],
                        start=False,
                        stop=True,
                    )

                    r = r_pool.tile([128, 2 * HC], f32, tag=f"r{c}")
                    _raw_act(
                        nc.scalar,
                        r[:, :],
                        psV[:, :],
                        mybir.ActivationFunctionType.Rsqrt,
                        eps_t[0:128, 0:1],
                        1.0,
                        mybir,
                    )

                    if i > 0:
                        p0 = ch.prev_sp[0].start
                        wv = W[:, p0 : p0 + 2 * LC].rearrange(
                            "p (b k) -> p b k", b=2
                        )[:, :, 0:HC]
                        dv = ch.prev_D[:, :].rearrange("p (b k) -> p b k", b=2)
                        rv = ch.prev_r[:, :].rearrange("p (b k) -> p b k", b=2)
                        nc.vector.tensor_mul(out=wv, in0=dv, in1=rv)
                        mul_done_pairs[c] = i - 1
                        if c == 1:
                            emit_out_dma(2 * min(mul_done_pairs))

                    ch.prev_D = psD
                    ch.prev_V = psV
                    ch.prev_r = r
                    ch.prev_sp = (s0, s1)

            # epilogue: flush last pair muls, then the 64-row tail block per chain
            for ch in chains:
                p0 = ch.prev_sp[0].start
                wv = W[:, p0 : p0 + 2 * LC].rearrange("p (b k) -> p b k", b=2)[
                    :, :, 0:HC
                ]
                dv = ch.prev_D[:, :].rearrange("p (b k) -> p b k", b=2)
                rv = ch.prev_r[:, :].rearrange("p (b k) -> p b k", b=2)
                nc.vector.tensor_mul(out=wv, in0=dv, in1=rv)
            emit_out_dma(NPAIR * 2)

            for ch in chains:
                c = ch.c
                st = col(64, c)
                nc.vector.tensor_copy(out=W[0:1, st], in_=ch.prev_D[0:1, HC : 2 * HC])
                psDt = psD_pool.tile([128, 2 * HC], f32, tag=f"psD{c}")
                nc.tensor.matmul(psDt[:, 0:HC], LD_s, W[:, st], start=True, stop=True)
                d2t = d2_pool.tile([128, 2 * HC], f32r, tag=f"d2{c}")
                _raw_act(
                    nc.scalar,
                    d2t[:, 0:HC],
                    psDt[:, 0:HC],
                    mybir.ActivationFunctionType.Square,
                    zero_t[0:128, 0:1],
                    SQ_SCALE,
                    mybir,
                )
                nc.vector.tensor_copy(
                    out=d2t[0:1, 0:HC], in_=ch.prev_V[0:1, HC : 2 * HC]
                )
                psVt = psV_pool.tile([128, 2 * HC], f32, tag=f"psV{c}")
                nc.tensor.matmul(
                    psVt[:, 0:HC], LV_s, d2t[:, 0:HC], start=True, stop=True
                )
                rt = r_pool.tile([128, 2 * HC], f32, tag=f"r{c}")
                _raw_act(
                    nc.scalar,
                    rt[:, 0:HC],
                    psVt[:, 0:HC],
                    mybir.ActivationFunctionType.Rsqrt,
                    eps_t[0:128, 0:1],
                    1.0,
                    mybir,
                )
                nc.vector.tensor_mul(
                    out=W[:, st], in0=psDt[:, 0:HC], in1=rt[:, 0:HC]
                )
            emit_out_dma(NBLK)

    nc.compile()
    _PROGRAM_CACHE["nc"] = nc
    return nc


def kernel(x: np.ndarray, m: np.ndarray, var: np.ndarray) -> np.ndarray:
    from concourse.bass_utils import run_bass_kernel_spmd

    x = np.asarray(x, dtype=_f32)
    m = np.ascontiguousarray(np.asarray(m, dtype=_f32))
    var = np.ascontiguousarray(np.asarray(var, dtype=_f32))
    assert x.shape == (N_ROWS, L_FULL), x.shape

    nc = _build_program()
    mats = _build_mats()

    in_maps = []
    for c in range(N_CORES):
        sl = slice(c * LC, (c + 1) * LC)
        in_maps.append(
            {
                "xp": _pretile(np.ascontiguousarray(x[:, sl]), m[sl]),
                "var": np.ascontiguousarray(var[sl]),
                "mats": mats.astype(_bf16),
                "matsv": mats,
            }
        )

    res = run_bass_kernel_spmd(nc, in_maps, core_ids=list(range(N_CORES)))
    out = np.concatenate(
        [_unpretile(res.results[c]["op"]) for c in range(N_CORES)], axis=1
    )
    return out.astype(_f32, copy=False)
